# revision 7
# baseline (speedup 1.0000x reference)
"""Trainium2 Bass kernel for the BaselinePointerHead problem.

Strategy
--------
- Shard batch B=16 across 8 NeuronCores (2 batches per core).
- Host (cheap, O(B) work): history-summary gather, query projection,
  RoPE rotation of the single query, and folding of (query x wk x scale
  x ln_g) into a per-batch combined weight matrix wAB so the device
  attention logit becomes
      l_att[s] = sum_d trig[d,s] * (xhat[s,:] @ wAB[b,d,:])
  with trig rows 0..511 = cos(pos*f), rows 512..1023 = sin(pos*f).
- Device (heavy, O(B*S*D^2)): layernorm of x_suf, two [S,1024]x[1024,1024]
  fp16 matmuls (wAB + u_w1), RoPE trig via range-reduced Sin activation,
  gelu, and per-position reductions via M=1 matmuls accumulated in PSUM.
- Host epilogue: + u_b2[role], mask -> -inf, argmax.
"""

import math

import numpy as np

B, S, M_CTX = 16, 4096, 200
D_EMB = 1024
D_MODEL = 1024
NUM_ROLES = 4
ROLE_DIM = 64
ROPE_BASE = 10000.0
LN_EPS = 1e-5
N_CORES = 8
BPC = B // N_CORES  # batches per core

MAGIC = 12582912.0  # 1.5 * 2**23, f32 round-to-int trick
TWOPI = float(2.0 * np.pi)
INV2PI = float(1.0 / TWOPI)

_PROGRAM_CACHE = {}


def _build_program():
    import concourse.bass as bass
    import concourse.mybir as mybir
    import concourse.tile as tile

    F32 = mybir.dt.float32
    FP16 = mybir.dt.float16
    AF = mybir.ActivationFunctionType
    ALU = mybir.AluOpType

    nc = bass.Bass("TRN2", target_bir_lowering=False, debug=False,
                   num_devices=N_CORES)

    x_d = nc.dram_tensor("x", [BPC, S, D_EMB], F32, kind="ExternalInput")
    pos_d = nc.dram_tensor("pos", [BPC, S], F32, kind="ExternalInput")
    wabt_d = nc.dram_tensor("wabt", [BPC, D_EMB, D_MODEL], FP16,
                            kind="ExternalInput")
    uw1t_d = nc.dram_tensor("uw1t", [D_EMB, D_MODEL], FP16,
                            kind="ExternalInput")
    cons_d = nc.dram_tensor("cons", [128, 16], F32, kind="ExternalInput")
    ub1_d = nc.dram_tensor("ub1", [128, 8], F32, kind="ExternalInput")
    vv_d = nc.dram_tensor("vv", [BPC, 128, 8], FP16, kind="ExternalInput")
    out_d = nc.dram_tensor("out", [BPC, S], F32, kind="ExternalOutput")

    EC = D_EMB // 128   # 8 contraction chunks
    DC = D_MODEL // 128  # 8 output chunks
    SC = 512             # s-tile width
    NSC = S // SC        # 8 s-chunks per batch
    NSUB = SC // 128     # 4 x-subtiles per s-chunk

    with tile.TileContext(nc) as tc:
        with (
            tc.tile_pool(name="consts", bufs=1) as consts,
            tc.tile_pool(name="weights", bufs=1) as weights,
            tc.tile_pool(name="xp", bufs=4) as xp,
            tc.tile_pool(name="stats", bufs=6) as stats,
            tc.tile_pool(name="xhp", bufs=5) as xhp,
            tc.tile_pool(name="xtp", bufs=2) as xtp,
            tc.tile_pool(name="posp", bufs=2) as posp,
            tc.tile_pool(name="thp", bufs=3) as thp,
            tc.tile_pool(name="trigp", bufs=3) as trigp,
            tc.tile_pool(name="pp", bufs=3) as pp,
            tc.tile_pool(name="hp", bufs=3) as hp,
            tc.tile_pool(name="orow", bufs=2) as orowp,
            tc.tile_pool(name="mmps", bufs=4, space="PSUM") as mmps,
            tc.tile_pool(name="lps", bufs=2, space="PSUM") as lps,
        ):
            # ---- constants / weights (loaded once) ----
            ones_c = consts.tile([128, 1], FP16, tag="ones")
            nc.vector.memset(ones_c[:], 1.0)
            zero_c = consts.tile([128, 1], F32, tag="zero")
            nc.vector.memset(zero_c[:], 0.0)
            eps_c = consts.tile([128, 1], F32, tag="eps")
            nc.vector.memset(eps_c[:], LN_EPS)
            cons = consts.tile([128, 16], F32, tag="cons")
            nc.sync.dma_start(cons[:], cons_d[:])
            ub1 = consts.tile([128, 8], F32, tag="ub1")
            nc.sync.dma_start(ub1[:], ub1_d[:])

            wab_sb = {}
            for b in range(BPC):
                for e in range(EC):
                    t = weights.tile([128, D_MODEL], FP16, tag=f"wab{b}_{e}")
                    nc.sync.dma_start(t[:], wabt_d[b, e * 128:(e + 1) * 128, :])
                    wab_sb[(b, e)] = t
            uw1_sb = {}
            for e in range(EC):
                t = weights.tile([128, D_MODEL], FP16, tag=f"uw1{e}")
                nc.sync.dma_start(t[:], uw1t_d[e * 128:(e + 1) * 128, :])
                uw1_sb[e] = t
            vv_sb = {}
            for b in range(BPC):
                t = weights.tile([128, 8], FP16, tag=f"vv{b}")
                nc.sync.dma_start(t[:], vv_d[b, :, :])
                vv_sb[b] = t

            # ---- main loop ----
            for b in range(BPC):
                for sc in range(NSC):
                    s0 = sc * SC
                    # pos row broadcast across partitions
                    posb = posp.tile([128, SC], F32, tag="posb")
                    src = pos_d[b, s0:s0 + SC]
                    bc = bass.AP(tensor=src.tensor, offset=src.offset,
                                 ap=[[0, 128]] + list(src.ap))
                    nc.sync.dma_start(out=posb[:], in_=bc)

                    # load + LN + transpose -> xts[e] = xhat^T chunks [e128, SC]
                    xts = [xtp.tile([128, SC], FP16, tag=f"xt{e}",
                                    name=f"xt{e}")
                           for e in range(EC)]
                    for i in range(NSUB):
                        r0 = s0 + i * 128
                        xt_i = xp.tile([128, D_EMB], F32, tag="x")
                        nc.sync.dma_start(xt_i[:], x_d[b, r0:r0 + 128, :])
                        st = stats.tile([128, 2, 6], F32, tag="st")
                        xg = xt_i[:].rearrange("p (g d) -> p g d", g=2)
                        for g in range(2):
                            nc.vector.bn_stats(st[:, g, :], xg[:, g, :])
                        mv = stats.tile([128, 2], F32, tag="mv")
                        nc.vector.bn_aggr(mv[:], st[:])
                        sq = stats.tile([128, 1], F32, tag="sq")
                        nc.scalar.activation(sq[:], mv[:, 1:2], AF.Sqrt,
                                             bias=eps_c[:], scale=1.0)
                        rstd = stats.tile([128, 1], F32, tag="rstd")
                        nc.vector.reciprocal(rstd[:], sq[:])
                        nmu = stats.tile([128, 1], F32, tag="nmu")
                        nc.vector.tensor_scalar(nmu[:], mv[:, 0:1], rstd[:],
                                                -1.0, ALU.mult, ALU.mult)
                        xh = xhp.tile([128, D_EMB], FP16, tag="xh")
                        nc.scalar.activation(xh[:], xt_i[:], AF.Identity,
                                             bias=nmu[:], scale=rstd[:])
                        for e in range(EC):
                            nc.scalar.dma_start_transpose(
                                xts[e][:, i * 128:(i + 1) * 128],
                                xh[:, e * 128:(e + 1) * 128])

                    lp = lps.tile([1, SC], F32, tag="lp")
                    n_acc = 0
                    # attention half: trig + wAB matmul + P + ones-reduce
                    for dc in range(DC):
                        th = thp.tile([128, SC], F32, tag="th")
                        nc.vector.tensor_scalar(th[:], posb[:],
                                                cons[:, dc:dc + 1],
                                                cons[:, 8 + dc:9 + dc],
                                                ALU.mult, ALU.add)
                        t2 = thp.tile([128, SC], F32, tag="t2")
                        nc.vector.tensor_scalar(t2[:], th[:], INV2PI, MAGIC,
                                                ALU.mult, ALU.add)
                        nc.gpsimd.tensor_scalar(t2[:], t2[:], MAGIC, -TWOPI,
                                                ALU.subtract, ALU.mult)
                        nc.gpsimd.tensor_tensor(out=th[:], in0=th[:],
                                                in1=t2[:], op=ALU.add)
                        trig = trigp.tile([128, SC], F32, tag="trig")
                        nc.scalar.activation(trig[:], th[:], AF.Sin,
                                             bias=zero_c[:], scale=1.0)
                        pm = mmps.tile([128, SC], F32, tag="pm")
                        for e in range(EC):
                            nc.tensor.matmul(
                                pm[:],
                                wab_sb[(b, e)][:, dc * 128:(dc + 1) * 128],
                                xts[e][:],
                                start=(e == 0), stop=(e == EC - 1))
                        pt = pp.tile([128, SC], FP16, tag="p")
                        nc.vector.tensor_tensor(out=pt[:], in0=pm[:],
                                                in1=trig[:], op=ALU.mult)
                        nc.tensor.matmul(lp[:], ones_c[:], pt[:],
                                         start=(n_acc == 0), stop=False)
                        n_acc += 1
                    # unary half: u_w1 matmul + gelu + v-reduce
                    for dc in range(DC):
                        pm = mmps.tile([128, SC], F32, tag="pm")
                        for e in range(EC):
                            nc.tensor.matmul(
                                pm[:],
                                uw1_sb[e][:, dc * 128:(dc + 1) * 128],
                                xts[e][:],
                                start=(e == 0), stop=(e == EC - 1))
                        h = hp.tile([128, SC], FP16, tag="h")
                        nc.scalar.activation(h[:], pm[:], AF.Gelu,
                                             bias=ub1[:, dc:dc + 1], scale=1.0)
                        nc.tensor.matmul(lp[:], vv_sb[b][:, dc:dc + 1], h[:],
                                         start=False, stop=(dc == DC - 1))
                    orow = orowp.tile([1, SC], F32, tag="orow")
                    nc.vector.tensor_copy(orow[:], lp[:])
                    nc.sync.dma_start(out_d[b, s0:s0 + SC], orow[:])

    _legalize_waits(nc)
    return nc


def _legalize_waits(nc, max_waits=1):
    """walrus on this stack accepts at most one sync-wait per instruction;
    move excess waits onto preceding same-engine NoOps."""
    import concourse.mybir as mybir

    ctr = 0
    for f in nc.m.functions:
        for bb in f.blocks:
            new_insts = []
            for inst in bb.instructions:
                si = inst.sync_info
                if si is not None and si.on_wait and len(si.on_wait) > max_waits:
                    waits = list(si.on_wait)
                    chunks = [waits[i:i + max_waits]
                              for i in range(0, len(waits), max_waits)]
                    for chunk in chunks[:-1]:
                        ctr += 1
                        nop = mybir.InstNoOp(name=f"waitnop-{ctr}", ins=[],
                                             outs=[])
                        nop.engine = inst.engine
                        nop.sync_info = mybir.SyncInfo(on_wait=list(chunk),
                                                       on_update=[])
                        new_insts.append(nop)
                    si.on_wait = chunks[-1]
                new_insts.append(inst)
            bb.instructions[:] = new_insts


def _host_prep(inputs):
    """All O(B)-sized math: summary, query, RoPE(q), weight folding."""
    f32 = np.float32
    x_ctx = np.asarray(inputs["x_ctx"], f32)
    pos_suf = np.asarray(inputs["pos_suf"])
    role_ctx = np.asarray(inputs["role_ctx"])
    role_tgt = np.asarray(inputs["role_tgt"])
    mask_ctx = np.asarray(inputs["mask_ctx"])
    ln_g = np.asarray(inputs["ln_g"], f32)
    ln_b = np.asarray(inputs["ln_b"], f32)
    hist_role_emb = np.asarray(inputs["hist_role_emb"], f32)
    tgt_role_emb = np.asarray(inputs["tgt_role_emb"], f32)
    start_state = np.asarray(inputs["start_state"], f32)
    wq = np.asarray(inputs["wq"], f32)
    wk = np.asarray(inputs["wk"], f32)
    u_w1 = np.asarray(inputs["u_w1"], f32)
    u_b1 = np.asarray(inputs["u_b1"], f32)
    u_w2 = np.asarray(inputs["u_w2"], f32)
    u_b2 = np.asarray(inputs["u_b2"], f32)

    lengths = mask_ctx.astype(np.int64).sum(1)
    last_idx = np.clip(lengths - 1, 0, None)
    last_x = x_ctx[np.arange(B), last_idx]
    last_role = role_ctx[np.arange(B), last_idx]
    cand = np.concatenate([last_x, hist_role_emb[last_role]], -1)
    summary = np.where((lengths > 0)[:, None], cand,
                       start_state[None, :]).astype(f32)
    q_in = np.concatenate([summary, tgt_role_emb[role_tgt]], -1).astype(f32)
    q = (q_in @ wq.T).astype(f32)                      # [B, D_MODEL]

    half = D_MODEL // 2
    inv_freq = (1.0 / (ROPE_BASE ** (np.arange(0, D_MODEL, 2,
                dtype=f32) / D_MODEL))).astype(f32)    # [512]
    anchor = pos_suf[:, :1].astype(f32) - 1.0          # [B, 1]
    ang = anchor * inv_freq[None, :]
    qc, qs = np.cos(ang).astype(f32), np.sin(ang).astype(f32)
    q1, q2 = q[:, :half], q[:, half:]
    qr1 = q1 * qc - q2 * qs
    qr2 = q1 * qs + q2 * qc

    scale = f32(1.0 / math.sqrt(D_MODEL))
    wk1, wk2 = wk[:half], wk[half:]                    # [512, D_EMB]
    wA = (qr1[:, :, None] * wk1[None] + qr2[:, :, None] * wk2[None]) * scale
    wB = (qr2[:, :, None] * wk1[None] - qr1[:, :, None] * wk2[None]) * scale
    wAB = (np.concatenate([wA, wB], axis=1) * ln_g[None, None, :]).astype(f32)
    # transpose to [e, d] (lhsT layout), fp16
    wabt = np.ascontiguousarray(wAB.transpose(0, 2, 1)).astype(np.float16)

    uw1g = (u_w1 * ln_g[None, :]).astype(f32)
    uw1t = np.ascontiguousarray(uw1g.T).astype(np.float16)  # [e, d]
    ub1e = (u_b1 + u_w1 @ ln_b).astype(f32)            # [1024]
    # attention-path layernorm-bias correction: zero when ln_b == 0.
    cab = wAB @ ln_b                                   # [B, 1024]
    assert np.abs(cab).max() < 1e-30, (
        "non-zero ln_b correction not supported by the device program")

    v = u_w2[role_tgt].astype(np.float16)              # [B, 1024]
    vb2 = u_b2[role_tgt].astype(f32)                   # [B]

    f_ext = np.concatenate([inv_freq, inv_freq])       # [1024]
    phase = np.concatenate([np.full(half, np.pi / 2, f32),
                            np.zeros(half, f32)])
    cons = np.zeros((128, 16), f32)
    cons[:, 0:8] = f_ext.reshape(8, 128).T
    cons[:, 8:16] = phase.reshape(8, 128).T
    ub1c = np.ascontiguousarray(ub1e.reshape(8, 128).T)  # [128, 8]
    vvc = np.ascontiguousarray(
        v.reshape(B, 8, 128).transpose(0, 2, 1))       # [B, 128, 8]

    pos_f = pos_suf.astype(f32)
    return dict(wabt=wabt, uw1t=uw1t, cons=cons, ub1c=ub1c, vvc=vvc,
                vb2=vb2, pos_f=pos_f)


def kernel(**inputs):
    from concourse.bass_utils import run_bass_kernel_spmd

    x_suf = np.asarray(inputs["x_suf"], np.float32)
    mask_suf = np.asarray(inputs["mask_suf"])
    role_tgt = np.asarray(inputs["role_tgt"])

    hp = _host_prep(inputs)

    if "prog" not in _PROGRAM_CACHE:
        _PROGRAM_CACHE["prog"] = _build_program()
    nc = _PROGRAM_CACHE["prog"]

    in_maps = []
    for c in range(N_CORES):
        bs = slice(c * BPC, (c + 1) * BPC)
        in_maps.append({
            "x": np.ascontiguousarray(x_suf[bs]),
            "pos": np.ascontiguousarray(hp["pos_f"][bs]),
            "wabt": np.ascontiguousarray(hp["wabt"][bs]),
            "uw1t": hp["uw1t"],
            "cons": hp["cons"],
            "ub1": hp["ub1c"],
            "vv": np.ascontiguousarray(hp["vvc"][bs]),
        })

    res = run_bass_kernel_spmd(nc, in_maps, list(range(N_CORES)))
    logits = np.empty((B, S), np.float32)
    for c in range(N_CORES):
        logits[c * BPC:(c + 1) * BPC] = res.results[c]["out"]

    logits += hp["vb2"][:, None]
    logits = np.where(mask_suf == 0, np.float32(-np.inf), logits)
    logits = logits.astype(np.float32)
    arg = np.argmax(logits, axis=-1).astype(np.int32)
    return logits, arg


# revision 12
# speedup vs baseline: 2.2963x; 2.2963x over previous
"""Trainium2 Bass kernel for the BaselinePointerHead problem.

Strategy
--------
- Shard batch B=16 across 8 NeuronCores (2 batches per core).
- Host (cheap, O(B) work): history-summary gather, query projection,
  RoPE rotation of the single query, and folding of (query x wk x scale
  x ln_g) into a per-batch combined weight matrix wAB so the device
  attention logit becomes
      l_att[s] = sum_d trig[d,s] * (xhat[s,:] @ wAB[b,d,:])
  with trig rows 0..511 = cos(pos*f), rows 512..1023 = sin(pos*f).
- Device (heavy, O(B*S*D^2)): layernorm of x_suf, two [S,1024]x[1024,1024]
  fp16 matmuls (wAB + u_w1), RoPE trig via range-reduced Sin activation,
  gelu, and per-position reductions via M=1 matmuls accumulated in PSUM.
- Host epilogue: + u_b2[role], mask -> -inf, argmax.
"""

import math

import numpy as np

B, S, M_CTX = 16, 4096, 200
D_EMB = 1024
D_MODEL = 1024
NUM_ROLES = 4
ROLE_DIM = 64
ROPE_BASE = 10000.0
LN_EPS = 1e-5
N_CORES = 8
BPC = B // N_CORES  # batches per core

MAGIC = 12582912.0  # 1.5 * 2**23, f32 round-to-int trick
TWOPI = float(2.0 * np.pi)
INV2PI = float(1.0 / TWOPI)

_PROGRAM_CACHE = {}


def _build_program():
    import concourse.bass as bass
    import concourse.mybir as mybir
    import concourse.tile as tile
    from concourse.masks import make_identity

    F32 = mybir.dt.float32
    FP16 = mybir.dt.float16
    AF = mybir.ActivationFunctionType
    ALU = mybir.AluOpType

    nc = bass.Bass("TRN2", target_bir_lowering=False, debug=False,
                   num_devices=N_CORES)

    x_d = nc.dram_tensor("x", [BPC, S, D_EMB], F32, kind="ExternalInput")
    pos_d = nc.dram_tensor("pos", [BPC, S], F32, kind="ExternalInput")
    wabt_d = nc.dram_tensor("wabt", [BPC, D_EMB, D_MODEL], FP16,
                            kind="ExternalInput")
    uw1t_d = nc.dram_tensor("uw1t", [D_EMB, D_MODEL], FP16,
                            kind="ExternalInput")
    cons_d = nc.dram_tensor("cons", [128, 16], F32, kind="ExternalInput")
    ub1_d = nc.dram_tensor("ub1", [128, 8], F32, kind="ExternalInput")
    vv_d = nc.dram_tensor("vv", [BPC, 128, 8], FP16, kind="ExternalInput")
    out_d = nc.dram_tensor("out", [BPC, S], F32, kind="ExternalOutput")

    EC = D_EMB // 128   # 8 contraction chunks
    DC = D_MODEL // 128  # 8 output chunks
    SC = 1024            # s-tile width
    NSC = S // SC        # 4 s-chunks per batch
    NSUB = SC // 128     # 8 x-subtiles per s-chunk
    HALF = 512           # matmul free-dim limit

    with tile.TileContext(nc) as tc:
        with (
            tc.tile_pool(name="consts", bufs=1) as consts,
            tc.tile_pool(name="weights", bufs=1) as weights,
            tc.tile_pool(name="xp", bufs=6) as xp,
            tc.tile_pool(name="stats", bufs=2) as stats,
            tc.tile_pool(name="xhp", bufs=6) as xhp,
            tc.tile_pool(name="xtp", bufs=2) as xtp,
            tc.tile_pool(name="posp", bufs=2) as posp,
            tc.tile_pool(name="thp", bufs=2) as thp,
            tc.tile_pool(name="trigp", bufs=2) as trigp,
            tc.tile_pool(name="pp", bufs=2) as pp,
            tc.tile_pool(name="hp", bufs=2) as hp,
            tc.tile_pool(name="orow", bufs=2) as orowp,
            tc.tile_pool(name="mmps", bufs=2, space="PSUM") as mmps,
            tc.tile_pool(name="tps", bufs=2, space="PSUM") as tps,
            tc.tile_pool(name="lps", bufs=1, space="PSUM") as lps,
        ):
            # ---- constants / weights (loaded once) ----
            ident_c = consts.tile([128, 128], FP16, tag="ident")
            make_identity(nc, ident_c[:])
            ones_c = consts.tile([128, 1], FP16, tag="ones")
            nc.vector.memset(ones_c[:], 1.0)
            zero_c = consts.tile([128, 1], F32, tag="zero")
            nc.vector.memset(zero_c[:], 0.0)
            halfpi_c = consts.tile([128, 1], F32, tag="halfpi")
            nc.vector.memset(halfpi_c[:], float(np.pi / 2))
            eps_c = consts.tile([128, 1], F32, tag="eps")
            nc.vector.memset(eps_c[:], LN_EPS)
            cons = consts.tile([128, 16], F32, tag="cons")
            nc.sync.dma_start(cons[:], cons_d[:])
            ub1 = consts.tile([128, 8], F32, tag="ub1")
            nc.sync.dma_start(ub1[:], ub1_d[:])

            wab_sb = {}
            for b in range(BPC):
                for e in range(EC):
                    t = weights.tile([128, D_MODEL], FP16, tag=f"wab{b}_{e}")
                    nc.sync.dma_start(t[:], wabt_d[b, e * 128:(e + 1) * 128, :])
                    wab_sb[(b, e)] = t
            uw1_sb = {}
            for e in range(EC):
                t = weights.tile([128, D_MODEL], FP16, tag=f"uw1{e}")
                nc.sync.dma_start(t[:], uw1t_d[e * 128:(e + 1) * 128, :])
                uw1_sb[e] = t
            vv_sb = {}
            for b in range(BPC):
                t = weights.tile([128, 8], FP16, tag=f"vv{b}")
                nc.sync.dma_start(t[:], vv_d[b, :, :])
                vv_sb[b] = t

            # ---- main loop ----
            for b in range(BPC):
                for sc in range(NSC):
                    s0 = sc * SC
                    # pos row broadcast across partitions
                    posb = posp.tile([128, SC], F32, tag="posb")
                    src = pos_d[b, s0:s0 + SC]
                    bc = bass.AP(tensor=src.tensor, offset=src.offset,
                                 ap=[[0, 128]] + list(src.ap))
                    nc.sync.dma_start(out=posb[:], in_=bc)

                    # load + LN + transpose in two groups of 4 subtiles
                    xts = [xtp.tile([128, SC], FP16, tag=f"xt{e}",
                                    name=f"xt{e}") for e in range(EC)]
                    for ih in range(2):
                        xtiles = []
                        mv_all = stats.tile([128, 4, 2], F32, tag="mv",
                                            name="mv")
                        for i4 in range(4):
                            i = ih * 4 + i4
                            r0 = s0 + i * 128
                            xt_i = xp.tile([128, D_EMB], F32, tag="x",
                                           name="x")
                            nc.sync.dma_start(xt_i[:], x_d[b, r0:r0 + 128, :])
                            st = stats.tile([128, 2, 6], F32, tag="st",
                                            name="st")
                            xg = xt_i[:].rearrange("p (g d) -> p g d", g=2)
                            for g in range(2):
                                nc.vector.bn_stats(st[:, g, :], xg[:, g, :])
                            nc.vector.bn_aggr(mv_all[:, i4, :], st[:])
                            xtiles.append(xt_i)
                        rstd_all = stats.tile([128, 4], F32, tag="rstd",
                                              name="rstd")
                        nc.scalar.activation(rstd_all[:], mv_all[:, :, 1],
                                             AF.Sqrt, bias=eps_c[:], scale=1.0)
                        nc.vector.reciprocal(rstd_all[:], rstd_all[:])
                        nmu_all = stats.tile([128, 4], F32, tag="nmu",
                                             name="nmu")
                        nc.vector.tensor_tensor(out=nmu_all[:],
                                                in0=mv_all[:, :, 0],
                                                in1=rstd_all[:], op=ALU.mult)
                        nc.vector.tensor_scalar(nmu_all[:], nmu_all[:], -1.0,
                                                None, ALU.mult)
                        xhs = []
                        for i4 in range(4):
                            xh = xhp.tile([128, D_EMB], FP16, tag="xh",
                                          name="xh")
                            nc.scalar.activation(xh[:], xtiles[i4][:],
                                                 AF.Identity,
                                                 bias=nmu_all[:, i4:i4 + 1],
                                                 scale=rstd_all[:, i4:i4 + 1])
                            xhs.append(xh)
                        for e in range(EC):
                            tp_t = tps.tile([128, 512], FP16, tag="tp",
                                            name="tp")
                            for i4 in range(4):
                                nc.tensor.transpose(
                                    tp_t[:, i4 * 128:(i4 + 1) * 128],
                                    xhs[i4][:, e * 128:(e + 1) * 128],
                                    ident_c[:])
                            dst = xts[e][:, ih * 512:(ih + 1) * 512]
                            nc.scalar.copy(dst, tp_t[:])

                    lpA = lps.tile([1, HALF], F32, tag="lpA", name="lpA")
                    lpB = lps.tile([1, HALF], F32, tag="lpB", name="lpB")
                    n_acc = 0
                    # attention half: trig + wAB matmul + P + ones-reduce
                    # f_ext chunks c and c+4 share frequencies:
                    #   dc=c   -> cos rows: sin(pi/2 - |thred|)
                    #   dc=c+4 -> sin rows: sin(thred)
                    trig_tiles = {}
                    for c in range(4):
                        th = thp.tile([128, SC], F32, tag="th", name="th")
                        nc.vector.tensor_scalar(th[:], posb[:],
                                                cons[:, c:c + 1], None,
                                                ALU.mult)
                        t2 = thp.tile([128, SC], F32, tag="t2", name="t2")
                        nc.vector.tensor_scalar(t2[:], th[:], INV2PI, MAGIC,
                                                ALU.mult, ALU.add)
                        nc.vector.tensor_scalar(t2[:], t2[:], MAGIC, -TWOPI,
                                                ALU.subtract, ALU.mult)
                        nc.vector.tensor_tensor(out=th[:], in0=th[:],
                                                in1=t2[:], op=ALU.add)
                        trig_s = trigp.tile([128, SC], F32, tag="trig_s",
                                            name="trig_s")
                        nc.scalar.activation(trig_s[:], th[:], AF.Sin,
                                             bias=zero_c[:], scale=1.0)
                        trig_tiles[c + 4] = trig_s
                        tha = thp.tile([128, SC], F32, tag="tha", name="tha")
                        nc.vector.tensor_scalar(tha[:], th[:], -1.0, None,
                                                ALU.mult)
                        nc.vector.tensor_tensor(out=tha[:], in0=tha[:],
                                                in1=th[:], op=ALU.max)
                        trig_c = trigp.tile([128, SC], F32, tag="trig_c",
                                            name="trig_c")
                        nc.scalar.activation(trig_c[:], tha[:], AF.Sin,
                                             bias=halfpi_c[:], scale=-1.0)
                        trig_tiles[c] = trig_c

                        for dc in (c, c + 4):
                            pm = mmps.tile([128, SC], F32, tag="pm",
                                           name="pm")
                            for e in range(EC):
                                lhs = wab_sb[(b, e)][:, dc * 128:(dc + 1) * 128]
                                nc.tensor.matmul(pm[:, 0:HALF], lhs,
                                                 xts[e][:, 0:HALF],
                                                 start=(e == 0),
                                                 stop=(e == EC - 1))
                                nc.tensor.matmul(pm[:, HALF:SC], lhs,
                                                 xts[e][:, HALF:SC],
                                                 start=(e == 0),
                                                 stop=(e == EC - 1))
                            pt = pp.tile([128, SC], FP16, tag="p", name="p")
                            nc.vector.tensor_tensor(out=pt[:], in0=pm[:],
                                                    in1=trig_tiles[dc][:],
                                                    op=ALU.mult)
                            nc.tensor.matmul(lpA[:], ones_c[:],
                                             pt[:, 0:HALF],
                                             start=(n_acc == 0), stop=False)
                            nc.tensor.matmul(lpB[:], ones_c[:],
                                             pt[:, HALF:SC],
                                             start=(n_acc == 0), stop=False)
                            n_acc += 1
                    # unary half: u_w1 matmul + gelu + v-reduce
                    for dc in range(DC):
                        pm = mmps.tile([128, SC], F32, tag="pm", name="pm")
                        for e in range(EC):
                            lhs = uw1_sb[e][:, dc * 128:(dc + 1) * 128]
                            nc.tensor.matmul(pm[:, 0:HALF], lhs,
                                             xts[e][:, 0:HALF],
                                             start=(e == 0),
                                             stop=(e == EC - 1))
                            nc.tensor.matmul(pm[:, HALF:SC], lhs,
                                             xts[e][:, HALF:SC],
                                             start=(e == 0),
                                             stop=(e == EC - 1))
                        h = hp.tile([128, SC], FP16, tag="h", name="h")
                        nc.scalar.activation(h[:], pm[:], AF.Gelu,
                                             bias=ub1[:, dc:dc + 1], scale=1.0)
                        last = (dc == DC - 1)
                        nc.tensor.matmul(lpA[:], vv_sb[b][:, dc:dc + 1],
                                         h[:, 0:HALF], start=False, stop=last)
                        nc.tensor.matmul(lpB[:], vv_sb[b][:, dc:dc + 1],
                                         h[:, HALF:SC], start=False, stop=last)
                    orow = orowp.tile([1, SC], F32, tag="orow", name="orow")
                    nc.vector.tensor_copy(orow[:, 0:HALF], lpA[:])
                    nc.vector.tensor_copy(orow[:, HALF:SC], lpB[:])
                    nc.sync.dma_start(out_d[b, s0:s0 + SC], orow[:])

    _legalize_waits(nc)
    return nc


def _legalize_waits(nc, max_waits=1):
    """walrus on this stack accepts at most one sync-wait per instruction;
    move excess waits onto preceding same-engine NoOps."""
    import concourse.mybir as mybir

    ctr = 0
    for f in nc.m.functions:
        for bb in f.blocks:
            new_insts = []
            for inst in bb.instructions:
                si = inst.sync_info
                if si is not None and si.on_wait and len(si.on_wait) > max_waits:
                    waits = list(si.on_wait)
                    chunks = [waits[i:i + max_waits]
                              for i in range(0, len(waits), max_waits)]
                    for chunk in chunks[:-1]:
                        ctr += 1
                        nop = mybir.InstNoOp(name=f"waitnop-{ctr}", ins=[],
                                             outs=[])
                        nop.engine = inst.engine
                        nop.sync_info = mybir.SyncInfo(on_wait=list(chunk),
                                                       on_update=[])
                        new_insts.append(nop)
                    si.on_wait = chunks[-1]
                new_insts.append(inst)
            bb.instructions[:] = new_insts


def _host_prep(inputs):
    """All O(B)-sized math: summary, query, RoPE(q), weight folding."""
    f32 = np.float32
    x_ctx = np.asarray(inputs["x_ctx"], f32)
    pos_suf = np.asarray(inputs["pos_suf"])
    role_ctx = np.asarray(inputs["role_ctx"])
    role_tgt = np.asarray(inputs["role_tgt"])
    mask_ctx = np.asarray(inputs["mask_ctx"])
    ln_g = np.asarray(inputs["ln_g"], f32)
    ln_b = np.asarray(inputs["ln_b"], f32)
    hist_role_emb = np.asarray(inputs["hist_role_emb"], f32)
    tgt_role_emb = np.asarray(inputs["tgt_role_emb"], f32)
    start_state = np.asarray(inputs["start_state"], f32)
    wq = np.asarray(inputs["wq"], f32)
    wk = np.asarray(inputs["wk"], f32)
    u_w1 = np.asarray(inputs["u_w1"], f32)
    u_b1 = np.asarray(inputs["u_b1"], f32)
    u_w2 = np.asarray(inputs["u_w2"], f32)
    u_b2 = np.asarray(inputs["u_b2"], f32)

    lengths = mask_ctx.astype(np.int64).sum(1)
    last_idx = np.clip(lengths - 1, 0, None)
    last_x = x_ctx[np.arange(B), last_idx]
    last_role = role_ctx[np.arange(B), last_idx]
    cand = np.concatenate([last_x, hist_role_emb[last_role]], -1)
    summary = np.where((lengths > 0)[:, None], cand,
                       start_state[None, :]).astype(f32)
    q_in = np.concatenate([summary, tgt_role_emb[role_tgt]], -1).astype(f32)
    q = (q_in @ wq.T).astype(f32)                      # [B, D_MODEL]

    half = D_MODEL // 2
    inv_freq = (1.0 / (ROPE_BASE ** (np.arange(0, D_MODEL, 2,
                dtype=f32) / D_MODEL))).astype(f32)    # [512]
    anchor = pos_suf[:, :1].astype(f32) - 1.0          # [B, 1]
    ang = anchor * inv_freq[None, :]
    qc, qs = np.cos(ang).astype(f32), np.sin(ang).astype(f32)
    q1, q2 = q[:, :half], q[:, half:]
    qr1 = q1 * qc - q2 * qs
    qr2 = q1 * qs + q2 * qc

    scale = f32(1.0 / math.sqrt(D_MODEL))
    wk1, wk2 = wk[:half], wk[half:]                    # [512, D_EMB]
    wA = (qr1[:, :, None] * wk1[None] + qr2[:, :, None] * wk2[None]) * scale
    wB = (qr2[:, :, None] * wk1[None] - qr1[:, :, None] * wk2[None]) * scale
    wAB = (np.concatenate([wA, wB], axis=1) * ln_g[None, None, :]).astype(f32)
    # transpose to [e, d] (lhsT layout), fp16
    wabt = np.ascontiguousarray(wAB.transpose(0, 2, 1)).astype(np.float16)

    uw1g = (u_w1 * ln_g[None, :]).astype(f32)
    uw1t = np.ascontiguousarray(uw1g.T).astype(np.float16)  # [e, d]
    ub1e = (u_b1 + u_w1 @ ln_b).astype(f32)            # [1024]
    # attention-path layernorm-bias correction: zero when ln_b == 0.
    cab = wAB @ ln_b                                   # [B, 1024]
    assert np.abs(cab).max() < 1e-30, (
        "non-zero ln_b correction not supported by the device program")

    v = u_w2[role_tgt].astype(np.float16)              # [B, 1024]
    vb2 = u_b2[role_tgt].astype(f32)                   # [B]

    f_ext = np.concatenate([inv_freq, inv_freq])       # [1024]
    phase = np.concatenate([np.full(half, np.pi / 2, f32),
                            np.zeros(half, f32)])
    cons = np.zeros((128, 16), f32)
    cons[:, 0:8] = f_ext.reshape(8, 128).T
    cons[:, 8:16] = phase.reshape(8, 128).T
    ub1c = np.ascontiguousarray(ub1e.reshape(8, 128).T)  # [128, 8]
    vvc = np.ascontiguousarray(
        v.reshape(B, 8, 128).transpose(0, 2, 1))       # [B, 128, 8]

    pos_f = pos_suf.astype(f32)
    return dict(wabt=wabt, uw1t=uw1t, cons=cons, ub1c=ub1c, vvc=vvc,
                vb2=vb2, pos_f=pos_f)


def kernel(**inputs):
    from concourse.bass_utils import run_bass_kernel_spmd

    x_suf = np.asarray(inputs["x_suf"], np.float32)
    mask_suf = np.asarray(inputs["mask_suf"])
    role_tgt = np.asarray(inputs["role_tgt"])

    hp = _host_prep(inputs)

    if "prog" not in _PROGRAM_CACHE:
        _PROGRAM_CACHE["prog"] = _build_program()
    nc = _PROGRAM_CACHE["prog"]

    in_maps = []
    for c in range(N_CORES):
        bs = slice(c * BPC, (c + 1) * BPC)
        in_maps.append({
            "x": np.ascontiguousarray(x_suf[bs]),
            "pos": np.ascontiguousarray(hp["pos_f"][bs]),
            "wabt": np.ascontiguousarray(hp["wabt"][bs]),
            "uw1t": hp["uw1t"],
            "cons": hp["cons"],
            "ub1": hp["ub1c"],
            "vv": np.ascontiguousarray(hp["vvc"][bs]),
        })

    res = run_bass_kernel_spmd(nc, in_maps, list(range(N_CORES)))
    logits = np.empty((B, S), np.float32)
    for c in range(N_CORES):
        logits[c * BPC:(c + 1) * BPC] = res.results[c]["out"]

    logits += hp["vb2"][:, None]
    logits = np.where(mask_suf == 0, np.float32(-np.inf), logits)
    logits = logits.astype(np.float32)
    arg = np.argmax(logits, axis=-1).astype(np.int32)
    return logits, arg


# revision 13
# speedup vs baseline: 2.2968x; 1.0002x over previous
"""Trainium2 Bass kernel for the BaselinePointerHead problem.

Strategy
--------
- Shard batch B=16 across 8 NeuronCores (2 batches per core).
- Host (cheap, O(B) work): history-summary gather, query projection,
  RoPE rotation of the single query, and folding of (query x wk x scale
  x ln_g) into a per-batch combined weight matrix wAB so the device
  attention logit becomes
      l_att[s] = sum_d trig[d,s] * (xhat[s,:] @ wAB[b,d,:])
  with trig rows 0..511 = cos(pos*f), rows 512..1023 = sin(pos*f).
- Device (heavy, O(B*S*D^2)): layernorm of x_suf, two [S,1024]x[1024,1024]
  fp16 matmuls (wAB + u_w1), RoPE trig via range-reduced Sin activation,
  gelu, and per-position reductions via M=1 matmuls accumulated in PSUM.
- Host epilogue: + u_b2[role], mask -> -inf, argmax.
"""

import math

import numpy as np

B, S, M_CTX = 16, 4096, 200
D_EMB = 1024
D_MODEL = 1024
NUM_ROLES = 4
ROLE_DIM = 64
ROPE_BASE = 10000.0
LN_EPS = 1e-5
N_CORES = 8
BPC = B // N_CORES  # batches per core

MAGIC = 12582912.0  # 1.5 * 2**23, f32 round-to-int trick
TWOPI = float(2.0 * np.pi)
INV2PI = float(1.0 / TWOPI)

_PROGRAM_CACHE = {}


def _build_program():
    import concourse.bass as bass
    import concourse.mybir as mybir
    import concourse.tile as tile
    from concourse.masks import make_identity

    F32 = mybir.dt.float32
    FP16 = mybir.dt.float16
    AF = mybir.ActivationFunctionType
    ALU = mybir.AluOpType

    nc = bass.Bass("TRN2", target_bir_lowering=False, debug=False,
                   num_devices=N_CORES)

    x_d = nc.dram_tensor("x", [BPC, S, D_EMB], F32, kind="ExternalInput")
    pos_d = nc.dram_tensor("pos", [BPC, S], F32, kind="ExternalInput")
    wabt_d = nc.dram_tensor("wabt", [BPC, D_EMB, D_MODEL], FP16,
                            kind="ExternalInput")
    uw1t_d = nc.dram_tensor("uw1t", [D_EMB, D_MODEL], FP16,
                            kind="ExternalInput")
    cons_d = nc.dram_tensor("cons", [128, 16], F32, kind="ExternalInput")
    ub1_d = nc.dram_tensor("ub1", [128, 8], F32, kind="ExternalInput")
    vv_d = nc.dram_tensor("vv", [BPC, 128, 8], FP16, kind="ExternalInput")
    out_d = nc.dram_tensor("out", [BPC, S], F32, kind="ExternalOutput")

    EC = D_EMB // 128   # 8 contraction chunks
    DC = D_MODEL // 128  # 8 output chunks
    SC = 1024            # s-tile width
    NSC = S // SC        # 4 s-chunks per batch
    NSUB = SC // 128     # 8 x-subtiles per s-chunk
    HALF = 512           # matmul free-dim limit

    with tile.TileContext(nc) as tc:
        with (
            tc.tile_pool(name="consts", bufs=1) as consts,
            tc.tile_pool(name="weights", bufs=1) as weights,
            tc.tile_pool(name="xp", bufs=6) as xp,
            tc.tile_pool(name="stats", bufs=2) as stats,
            tc.tile_pool(name="xhp", bufs=6) as xhp,
            tc.tile_pool(name="xtp", bufs=2) as xtp,
            tc.tile_pool(name="posp", bufs=2) as posp,
            tc.tile_pool(name="thp", bufs=2) as thp,
            tc.tile_pool(name="trigp", bufs=2) as trigp,
            tc.tile_pool(name="pp", bufs=2) as pp,
            tc.tile_pool(name="hp", bufs=2) as hp,
            tc.tile_pool(name="orow", bufs=2) as orowp,
            tc.tile_pool(name="mmps", bufs=2, space="PSUM") as mmps,
            tc.tile_pool(name="tps", bufs=2, space="PSUM") as tps,
            tc.tile_pool(name="lps", bufs=1, space="PSUM") as lps,
        ):
            # ---- constants / weights (loaded once) ----
            ident_c = consts.tile([128, 128], FP16, tag="ident")
            make_identity(nc, ident_c[:])
            ones_c = consts.tile([128, 1], FP16, tag="ones")
            nc.vector.memset(ones_c[:], 1.0)
            zero_c = consts.tile([128, 1], F32, tag="zero")
            nc.vector.memset(zero_c[:], 0.0)
            halfpi_c = consts.tile([128, 1], F32, tag="halfpi")
            nc.vector.memset(halfpi_c[:], float(np.pi / 2))
            eps_c = consts.tile([128, 1], F32, tag="eps")
            nc.vector.memset(eps_c[:], LN_EPS)
            cons = consts.tile([128, 16], F32, tag="cons")
            nc.sync.dma_start(cons[:], cons_d[:])
            ub1 = consts.tile([128, 8], F32, tag="ub1")
            nc.sync.dma_start(ub1[:], ub1_d[:])

            wab_sb = {}
            uw1_sb = {}
            vv_sb = {}
            def _load_wab(b):
                for e in range(EC):
                    t = weights.tile([128, D_MODEL], FP16, tag=f"wab{b}_{e}",
                                     name=f"wab{b}_{e}")
                    nc.sync.dma_start(t[:], wabt_d[b, e * 128:(e + 1) * 128, :])
                    wab_sb[(b, e)] = t
            _load_wab(0)
            for e in range(EC):
                t = weights.tile([128, D_MODEL], FP16, tag=f"uw1{e}",
                                 name=f"uw1{e}")
                nc.sync.dma_start(t[:], uw1t_d[e * 128:(e + 1) * 128, :])
                uw1_sb[e] = t
            for b in range(BPC):
                t = weights.tile([128, 8], FP16, tag=f"vv{b}", name=f"vv{b}")
                nc.sync.dma_start(t[:], vv_d[b, :, :])
                vv_sb[b] = t
            _load_wab(1)

            # ---- main loop ----
            for b in range(BPC):
                for sc in range(NSC):
                    s0 = sc * SC
                    # pos row broadcast across partitions
                    posb = posp.tile([128, SC], F32, tag="posb")
                    src = pos_d[b, s0:s0 + SC]
                    bc = bass.AP(tensor=src.tensor, offset=src.offset,
                                 ap=[[0, 128]] + list(src.ap))
                    nc.sync.dma_start(out=posb[:], in_=bc)

                    # load + LN + transpose in two groups of 4 subtiles
                    xts = [xtp.tile([128, SC], FP16, tag=f"xt{e}",
                                    name=f"xt{e}") for e in range(EC)]
                    for ih in range(2):
                        xtiles = []
                        mv_all = stats.tile([128, 4, 2], F32, tag="mv",
                                            name="mv")
                        for i4 in range(4):
                            i = ih * 4 + i4
                            r0 = s0 + i * 128
                            xt_i = xp.tile([128, D_EMB], F32, tag="x",
                                           name="x")
                            nc.sync.dma_start(xt_i[:], x_d[b, r0:r0 + 128, :])
                            st = stats.tile([128, 2, 6], F32, tag="st",
                                            name="st")
                            xg = xt_i[:].rearrange("p (g d) -> p g d", g=2)
                            for g in range(2):
                                nc.vector.bn_stats(st[:, g, :], xg[:, g, :])
                            nc.vector.bn_aggr(mv_all[:, i4, :], st[:])
                            xtiles.append(xt_i)
                        rstd_all = stats.tile([128, 4], F32, tag="rstd",
                                              name="rstd")
                        nc.scalar.activation(rstd_all[:], mv_all[:, :, 1],
                                             AF.Sqrt, bias=eps_c[:], scale=1.0)
                        nc.vector.reciprocal(rstd_all[:], rstd_all[:])
                        nmu_all = stats.tile([128, 4], F32, tag="nmu",
                                             name="nmu")
                        nc.vector.tensor_tensor(out=nmu_all[:],
                                                in0=mv_all[:, :, 0],
                                                in1=rstd_all[:], op=ALU.mult)
                        nc.vector.tensor_scalar(nmu_all[:], nmu_all[:], -1.0,
                                                None, ALU.mult)
                        xhs = []
                        for i4 in range(4):
                            xh = xhp.tile([128, D_EMB], FP16, tag="xh",
                                          name="xh")
                            nc.scalar.activation(xh[:], xtiles[i4][:],
                                                 AF.Identity,
                                                 bias=nmu_all[:, i4:i4 + 1],
                                                 scale=rstd_all[:, i4:i4 + 1])
                            xhs.append(xh)
                        for e in range(EC):
                            tp_t = tps.tile([128, 512], FP16, tag="tp",
                                            name="tp")
                            for i4 in range(4):
                                nc.tensor.transpose(
                                    tp_t[:, i4 * 128:(i4 + 1) * 128],
                                    xhs[i4][:, e * 128:(e + 1) * 128],
                                    ident_c[:])
                            dst = xts[e][:, ih * 512:(ih + 1) * 512]
                            nc.scalar.copy(dst, tp_t[:])

                    lpA = lps.tile([1, HALF], F32, tag="lpA", name="lpA")
                    lpB = lps.tile([1, HALF], F32, tag="lpB", name="lpB")
                    n_acc = 0
                    # attention half: trig + wAB matmul + P + ones-reduce
                    # f_ext chunks c and c+4 share frequencies:
                    #   dc=c   -> cos rows: sin(pi/2 - |thred|)
                    #   dc=c+4 -> sin rows: sin(thred)
                    trig_tiles = {}
                    for c in range(4):
                        th = thp.tile([128, SC], F32, tag="th", name="th")
                        nc.vector.tensor_scalar(th[:], posb[:],
                                                cons[:, c:c + 1], None,
                                                ALU.mult)
                        t2 = thp.tile([128, SC], F32, tag="t2", name="t2")
                        nc.vector.tensor_scalar(t2[:], th[:], INV2PI, MAGIC,
                                                ALU.mult, ALU.add)
                        nc.vector.tensor_scalar(t2[:], t2[:], MAGIC, -TWOPI,
                                                ALU.subtract, ALU.mult)
                        nc.vector.tensor_tensor(out=th[:], in0=th[:],
                                                in1=t2[:], op=ALU.add)
                        trig_s = trigp.tile([128, SC], F32, tag="trig_s",
                                            name="trig_s")
                        nc.scalar.activation(trig_s[:], th[:], AF.Sin,
                                             bias=zero_c[:], scale=1.0)
                        trig_tiles[c + 4] = trig_s
                        tha = thp.tile([128, SC], F32, tag="tha", name="tha")
                        nc.vector.tensor_scalar(tha[:], th[:], -1.0, None,
                                                ALU.mult)
                        nc.vector.tensor_tensor(out=tha[:], in0=tha[:],
                                                in1=th[:], op=ALU.max)
                        trig_c = trigp.tile([128, SC], F32, tag="trig_c",
                                            name="trig_c")
                        nc.scalar.activation(trig_c[:], tha[:], AF.Sin,
                                             bias=halfpi_c[:], scale=-1.0)
                        trig_tiles[c] = trig_c

                        for dc in (c, c + 4):
                            pm = mmps.tile([128, SC], F32, tag="pm",
                                           name="pm")
                            for e in range(EC):
                                lhs = wab_sb[(b, e)][:, dc * 128:(dc + 1) * 128]
                                nc.tensor.matmul(pm[:, 0:HALF], lhs,
                                                 xts[e][:, 0:HALF],
                                                 start=(e == 0),
                                                 stop=(e == EC - 1))
                                nc.tensor.matmul(pm[:, HALF:SC], lhs,
                                                 xts[e][:, HALF:SC],
                                                 start=(e == 0),
                                                 stop=(e == EC - 1))
                            pt = pp.tile([128, SC], FP16, tag="p", name="p")
                            with tc.high_priority(offset=3000):
                                nc.vector.tensor_tensor(out=pt[:], in0=pm[:],
                                                        in1=trig_tiles[dc][:],
                                                        op=ALU.mult)
                            nc.tensor.matmul(lpA[:], ones_c[:],
                                             pt[:, 0:HALF],
                                             start=(n_acc == 0), stop=False)
                            nc.tensor.matmul(lpB[:], ones_c[:],
                                             pt[:, HALF:SC],
                                             start=(n_acc == 0), stop=False)
                            n_acc += 1
                    # unary half: u_w1 matmul + gelu + v-reduce
                    for dc in range(DC):
                        pm = mmps.tile([128, SC], F32, tag="pm", name="pm")
                        for e in range(EC):
                            lhs = uw1_sb[e][:, dc * 128:(dc + 1) * 128]
                            nc.tensor.matmul(pm[:, 0:HALF], lhs,
                                             xts[e][:, 0:HALF],
                                             start=(e == 0),
                                             stop=(e == EC - 1))
                            nc.tensor.matmul(pm[:, HALF:SC], lhs,
                                             xts[e][:, HALF:SC],
                                             start=(e == 0),
                                             stop=(e == EC - 1))
                        h = hp.tile([128, SC], FP16, tag="h", name="h")
                        with tc.high_priority(offset=3000):
                            nc.scalar.activation(h[:], pm[:], AF.Gelu,
                                                 bias=ub1[:, dc:dc + 1],
                                                 scale=1.0)
                        last = (dc == DC - 1)
                        nc.tensor.matmul(lpA[:], vv_sb[b][:, dc:dc + 1],
                                         h[:, 0:HALF], start=False, stop=last)
                        nc.tensor.matmul(lpB[:], vv_sb[b][:, dc:dc + 1],
                                         h[:, HALF:SC], start=False, stop=last)
                    orow = orowp.tile([1, SC], F32, tag="orow", name="orow")
                    nc.vector.tensor_copy(orow[:, 0:HALF], lpA[:])
                    nc.vector.tensor_copy(orow[:, HALF:SC], lpB[:])
                    nc.sync.dma_start(out_d[b, s0:s0 + SC], orow[:])

    _legalize_waits(nc)
    return nc


def _legalize_waits(nc, max_waits=1):
    """walrus on this stack accepts at most one sync-wait per instruction;
    move excess waits onto preceding same-engine NoOps."""
    import concourse.mybir as mybir

    ctr = 0
    for f in nc.m.functions:
        for bb in f.blocks:
            new_insts = []
            for inst in bb.instructions:
                si = inst.sync_info
                if si is not None and si.on_wait and len(si.on_wait) > max_waits:
                    waits = list(si.on_wait)
                    chunks = [waits[i:i + max_waits]
                              for i in range(0, len(waits), max_waits)]
                    for chunk in chunks[:-1]:
                        ctr += 1
                        nop = mybir.InstNoOp(name=f"waitnop-{ctr}", ins=[],
                                             outs=[])
                        nop.engine = inst.engine
                        nop.sync_info = mybir.SyncInfo(on_wait=list(chunk),
                                                       on_update=[])
                        new_insts.append(nop)
                    si.on_wait = chunks[-1]
                new_insts.append(inst)
            bb.instructions[:] = new_insts


def _host_prep(inputs):
    """All O(B)-sized math: summary, query, RoPE(q), weight folding."""
    f32 = np.float32
    x_ctx = np.asarray(inputs["x_ctx"], f32)
    pos_suf = np.asarray(inputs["pos_suf"])
    role_ctx = np.asarray(inputs["role_ctx"])
    role_tgt = np.asarray(inputs["role_tgt"])
    mask_ctx = np.asarray(inputs["mask_ctx"])
    ln_g = np.asarray(inputs["ln_g"], f32)
    ln_b = np.asarray(inputs["ln_b"], f32)
    hist_role_emb = np.asarray(inputs["hist_role_emb"], f32)
    tgt_role_emb = np.asarray(inputs["tgt_role_emb"], f32)
    start_state = np.asarray(inputs["start_state"], f32)
    wq = np.asarray(inputs["wq"], f32)
    wk = np.asarray(inputs["wk"], f32)
    u_w1 = np.asarray(inputs["u_w1"], f32)
    u_b1 = np.asarray(inputs["u_b1"], f32)
    u_w2 = np.asarray(inputs["u_w2"], f32)
    u_b2 = np.asarray(inputs["u_b2"], f32)

    lengths = mask_ctx.astype(np.int64).sum(1)
    last_idx = np.clip(lengths - 1, 0, None)
    last_x = x_ctx[np.arange(B), last_idx]
    last_role = role_ctx[np.arange(B), last_idx]
    cand = np.concatenate([last_x, hist_role_emb[last_role]], -1)
    summary = np.where((lengths > 0)[:, None], cand,
                       start_state[None, :]).astype(f32)
    q_in = np.concatenate([summary, tgt_role_emb[role_tgt]], -1).astype(f32)
    q = (q_in @ wq.T).astype(f32)                      # [B, D_MODEL]

    half = D_MODEL // 2
    inv_freq = (1.0 / (ROPE_BASE ** (np.arange(0, D_MODEL, 2,
                dtype=f32) / D_MODEL))).astype(f32)    # [512]
    anchor = pos_suf[:, :1].astype(f32) - 1.0          # [B, 1]
    ang = anchor * inv_freq[None, :]
    qc, qs = np.cos(ang).astype(f32), np.sin(ang).astype(f32)
    q1, q2 = q[:, :half], q[:, half:]
    qr1 = q1 * qc - q2 * qs
    qr2 = q1 * qs + q2 * qc

    scale = f32(1.0 / math.sqrt(D_MODEL))
    wk1, wk2 = wk[:half], wk[half:]                    # [512, D_EMB]
    wA = (qr1[:, :, None] * wk1[None] + qr2[:, :, None] * wk2[None]) * scale
    wB = (qr2[:, :, None] * wk1[None] - qr1[:, :, None] * wk2[None]) * scale
    wAB = (np.concatenate([wA, wB], axis=1) * ln_g[None, None, :]).astype(f32)
    # transpose to [e, d] (lhsT layout), fp16
    wabt = np.ascontiguousarray(wAB.transpose(0, 2, 1)).astype(np.float16)

    uw1g = (u_w1 * ln_g[None, :]).astype(f32)
    uw1t = np.ascontiguousarray(uw1g.T).astype(np.float16)  # [e, d]
    ub1e = (u_b1 + u_w1 @ ln_b).astype(f32)            # [1024]
    # attention-path layernorm-bias correction: zero when ln_b == 0.
    cab = wAB @ ln_b                                   # [B, 1024]
    assert np.abs(cab).max() < 1e-30, (
        "non-zero ln_b correction not supported by the device program")

    v = u_w2[role_tgt].astype(np.float16)              # [B, 1024]
    vb2 = u_b2[role_tgt].astype(f32)                   # [B]

    f_ext = np.concatenate([inv_freq, inv_freq])       # [1024]
    phase = np.concatenate([np.full(half, np.pi / 2, f32),
                            np.zeros(half, f32)])
    cons = np.zeros((128, 16), f32)
    cons[:, 0:8] = f_ext.reshape(8, 128).T
    cons[:, 8:16] = phase.reshape(8, 128).T
    ub1c = np.ascontiguousarray(ub1e.reshape(8, 128).T)  # [128, 8]
    vvc = np.ascontiguousarray(
        v.reshape(B, 8, 128).transpose(0, 2, 1))       # [B, 128, 8]

    pos_f = pos_suf.astype(f32)
    return dict(wabt=wabt, uw1t=uw1t, cons=cons, ub1c=ub1c, vvc=vvc,
                vb2=vb2, pos_f=pos_f)


def kernel(**inputs):
    from concourse.bass_utils import run_bass_kernel_spmd

    x_suf = np.asarray(inputs["x_suf"], np.float32)
    mask_suf = np.asarray(inputs["mask_suf"])
    role_tgt = np.asarray(inputs["role_tgt"])

    hp = _host_prep(inputs)

    if "prog" not in _PROGRAM_CACHE:
        _PROGRAM_CACHE["prog"] = _build_program()
    nc = _PROGRAM_CACHE["prog"]

    in_maps = []
    for c in range(N_CORES):
        bs = slice(c * BPC, (c + 1) * BPC)
        in_maps.append({
            "x": np.ascontiguousarray(x_suf[bs]),
            "pos": np.ascontiguousarray(hp["pos_f"][bs]),
            "wabt": np.ascontiguousarray(hp["wabt"][bs]),
            "uw1t": hp["uw1t"],
            "cons": hp["cons"],
            "ub1": hp["ub1c"],
            "vv": np.ascontiguousarray(hp["vvc"][bs]),
        })

    res = run_bass_kernel_spmd(nc, in_maps, list(range(N_CORES)))
    logits = np.empty((B, S), np.float32)
    for c in range(N_CORES):
        logits[c * BPC:(c + 1) * BPC] = res.results[c]["out"]

    logits += hp["vb2"][:, None]
    logits = np.where(mask_suf == 0, np.float32(-np.inf), logits)
    logits = logits.astype(np.float32)
    arg = np.argmax(logits, axis=-1).astype(np.int32)
    return logits, arg


# revision 15
# speedup vs baseline: 2.4010x; 1.0454x over previous
"""Trainium2 Bass kernel for the BaselinePointerHead problem.

Strategy
--------
- Shard batch B=16 across 8 NeuronCores (2 batches per core).
- Host (cheap, O(B) work): history-summary gather, query projection,
  RoPE rotation of the single query, and folding of (query x wk x scale
  x ln_g) into a per-batch combined weight matrix wAB so the device
  attention logit becomes
      l_att[s] = sum_d trig[d,s] * (xhat[s,:] @ wAB[b,d,:])
  with trig rows 0..511 = cos(pos*f), rows 512..1023 = sin(pos*f).
- Device (heavy, O(B*S*D^2)): layernorm of x_suf, two [S,1024]x[1024,1024]
  fp16 matmuls (wAB + u_w1), RoPE trig via range-reduced Sin activation,
  gelu, and per-position reductions via M=1 matmuls accumulated in PSUM.
- Host epilogue: + u_b2[role], mask -> -inf, argmax.
"""

import math

import numpy as np

B, S, M_CTX = 16, 4096, 200
D_EMB = 1024
D_MODEL = 1024
NUM_ROLES = 4
ROLE_DIM = 64
ROPE_BASE = 10000.0
LN_EPS = 1e-5
N_CORES = 8
BPC = B // N_CORES  # batches per core

MAGIC = 12582912.0  # 1.5 * 2**23, f32 round-to-int trick
TWOPI = float(2.0 * np.pi)
INV2PI = float(1.0 / TWOPI)

_PROGRAM_CACHE = {}


def _build_program():
    import concourse.bass as bass
    import concourse.mybir as mybir
    import concourse.tile as tile
    from concourse.masks import make_identity

    F32 = mybir.dt.float32
    FP16 = mybir.dt.float16
    AF = mybir.ActivationFunctionType
    ALU = mybir.AluOpType

    nc = bass.Bass("TRN2", target_bir_lowering=False, debug=False,
                   num_devices=N_CORES)

    x_d = nc.dram_tensor("x", [BPC, S, D_EMB], F32, kind="ExternalInput")
    pos_d = nc.dram_tensor("pos", [BPC, S], F32, kind="ExternalInput")
    wabt_d = nc.dram_tensor("wabt", [BPC, D_EMB, D_MODEL], FP16,
                            kind="ExternalInput")
    uw1t_d = nc.dram_tensor("uw1t", [D_EMB, D_MODEL], FP16,
                            kind="ExternalInput")
    cons_d = nc.dram_tensor("cons", [128, 16], F32, kind="ExternalInput")
    ub1_d = nc.dram_tensor("ub1", [128, 8], F32, kind="ExternalInput")
    vv_d = nc.dram_tensor("vv", [BPC, 128, 8], FP16, kind="ExternalInput")
    out_d = nc.dram_tensor("out", [BPC, S], F32, kind="ExternalOutput")

    EC = D_EMB // 128   # 8 contraction chunks
    DC = D_MODEL // 128  # 8 output chunks
    SC = 1024            # s-tile width
    NSC = S // SC        # 4 s-chunks per batch
    NSUB = SC // 128     # 8 x-subtiles per s-chunk
    HALF = 512           # matmul free-dim limit

    with tile.TileContext(nc) as tc:
        with (
            tc.tile_pool(name="consts", bufs=1) as consts,
            tc.tile_pool(name="weights", bufs=1) as weights,
            tc.tile_pool(name="xp", bufs=6) as xp,
            tc.tile_pool(name="stats", bufs=2) as stats,
            tc.tile_pool(name="xhp", bufs=6) as xhp,
            tc.tile_pool(name="xtp", bufs=2) as xtp,
            tc.tile_pool(name="posp", bufs=2) as posp,
            tc.tile_pool(name="thp", bufs=2) as thp,
            tc.tile_pool(name="trigp", bufs=2) as trigp,
            tc.tile_pool(name="pp", bufs=2) as pp,
            tc.tile_pool(name="hp", bufs=2) as hp,
            tc.tile_pool(name="orow", bufs=2) as orowp,
            tc.tile_pool(name="mmps", bufs=2, space="PSUM") as mmps,
            tc.tile_pool(name="tps", bufs=2, space="PSUM") as tps,
            tc.tile_pool(name="lps", bufs=2, space="PSUM") as lps,
        ):
            # ---- constants / weights (loaded once) ----
            ident_c = consts.tile([128, 128], FP16, tag="ident")
            make_identity(nc, ident_c[:])
            ones_c = consts.tile([128, 1], FP16, tag="ones")
            nc.vector.memset(ones_c[:], 1.0)
            zero_c = consts.tile([128, 1], F32, tag="zero")
            nc.vector.memset(zero_c[:], 0.0)
            halfpi_c = consts.tile([128, 1], F32, tag="halfpi")
            nc.vector.memset(halfpi_c[:], float(np.pi / 2))
            eps_c = consts.tile([128, 1], F32, tag="eps")
            nc.vector.memset(eps_c[:], LN_EPS)
            cons = consts.tile([128, 16], F32, tag="cons")
            nc.sync.dma_start(cons[:], cons_d[:])
            ub1 = consts.tile([128, 8], F32, tag="ub1")
            nc.sync.dma_start(ub1[:], ub1_d[:])

            wab_sb = {}
            uw1_sb = {}
            vv_sb = {}
            def _load_wab(b):
                for e in range(EC):
                    t = weights.tile([128, D_MODEL], FP16, tag=f"wab{b}_{e}",
                                     name=f"wab{b}_{e}")
                    nc.sync.dma_start(t[:], wabt_d[b, e * 128:(e + 1) * 128, :])
                    wab_sb[(b, e)] = t
            _load_wab(0)
            for e in range(EC):
                t = weights.tile([128, D_MODEL], FP16, tag=f"uw1{e}",
                                 name=f"uw1{e}")
                nc.sync.dma_start(t[:], uw1t_d[e * 128:(e + 1) * 128, :])
                uw1_sb[e] = t
            for b in range(BPC):
                t = weights.tile([128, 8], FP16, tag=f"vv{b}", name=f"vv{b}")
                nc.sync.dma_start(t[:], vv_d[b, :, :])
                vv_sb[b] = t
            _load_wab(1)

            # ---- main loop ----
            from contextlib import nullcontext
            for b in range(BPC):
                for sc in range(NSC):
                    s0 = sc * SC
                    first = (b == 0 and sc == 0)
                    # pos row broadcast across partitions
                    posb = posp.tile([128, SC], F32, tag="posb")
                    src = pos_d[b, s0:s0 + SC]
                    bc = bass.AP(tensor=src.tensor, offset=src.offset,
                                 ap=[[0, 128]] + list(src.ap))
                    nc.sync.dma_start(out=posb[:], in_=bc)

                    # load + LN + transpose in two groups of 4 subtiles
                    xts = [xtp.tile([128, SC], FP16, tag=f"xt{e}",
                                    name=f"xt{e}") for e in range(EC)]
                    for ih in range(2):
                        xtiles = []
                        mv_all = stats.tile([128, 4, 2], F32, tag="mv",
                                            name="mv")
                        for i4 in range(4):
                            i = ih * 4 + i4
                            r0 = s0 + i * 128
                            xt_i = xp.tile([128, D_EMB], F32, tag="x",
                                           name="x")
                            with (tc.high_priority(offset=10 ** 6)
                                  if (first and ih == 0) else nullcontext()):
                                nc.sync.dma_start(xt_i[:],
                                                  x_d[b, r0:r0 + 128, :])
                            st = stats.tile([128, 2, 6], F32, tag="st",
                                            name="st")
                            xg = xt_i[:].rearrange("p (g d) -> p g d", g=2)
                            for g in range(2):
                                nc.vector.bn_stats(st[:, g, :], xg[:, g, :])
                            nc.vector.bn_aggr(mv_all[:, i4, :], st[:])
                            xtiles.append(xt_i)
                        rstd_all = stats.tile([128, 4], F32, tag="rstd",
                                              name="rstd")
                        nc.scalar.activation(rstd_all[:], mv_all[:, :, 1],
                                             AF.Sqrt, bias=eps_c[:], scale=1.0)
                        nc.vector.reciprocal(rstd_all[:], rstd_all[:])
                        nmu_all = stats.tile([128, 4], F32, tag="nmu",
                                             name="nmu")
                        nc.vector.tensor_tensor(out=nmu_all[:],
                                                in0=mv_all[:, :, 0],
                                                in1=rstd_all[:], op=ALU.mult)
                        nc.vector.tensor_scalar(nmu_all[:], nmu_all[:], -1.0,
                                                None, ALU.mult)
                        xhs = []
                        for i4 in range(4):
                            xh = xhp.tile([128, D_EMB], FP16, tag="xh",
                                          name="xh")
                            nc.scalar.activation(xh[:], xtiles[i4][:],
                                                 AF.Identity,
                                                 bias=nmu_all[:, i4:i4 + 1],
                                                 scale=rstd_all[:, i4:i4 + 1])
                            xhs.append(xh)
                        for e in range(EC):
                            tp_t = tps.tile([128, 512], FP16, tag="tp",
                                            name="tp")
                            for i4 in range(4):
                                nc.tensor.transpose(
                                    tp_t[:, i4 * 128:(i4 + 1) * 128],
                                    xhs[i4][:, e * 128:(e + 1) * 128],
                                    ident_c[:])
                            dst = xts[e][:, ih * 512:(ih + 1) * 512]
                            nc.scalar.copy(dst, tp_t[:])

                    lp = lps.tile([128, HALF], F32, tag="lp", name="lp")
                    lpA = lp[0:1, :]
                    lpB = lp[32:33, :]
                    n_acc = 0
                    # attention half: trig + wAB matmul + P + ones-reduce
                    # f_ext chunks c and c+4 share frequencies:
                    #   dc=c   -> cos rows: sin(pi/2 - |thred|)
                    #   dc=c+4 -> sin rows: sin(thred)
                    trig_tiles = {}
                    for c in range(4):
                        th = thp.tile([128, SC], F32, tag="th", name="th")
                        nc.vector.tensor_scalar(th[:], posb[:],
                                                cons[:, c:c + 1], None,
                                                ALU.mult)
                        t2 = thp.tile([128, SC], F32, tag="t2", name="t2")
                        nc.vector.tensor_scalar(t2[:], th[:], INV2PI, MAGIC,
                                                ALU.mult, ALU.add)
                        nc.vector.tensor_scalar(t2[:], t2[:], MAGIC, -TWOPI,
                                                ALU.subtract, ALU.mult)
                        nc.vector.tensor_tensor(out=th[:], in0=th[:],
                                                in1=t2[:], op=ALU.add)
                        trig_s = trigp.tile([128, SC], F32, tag="trig_s",
                                            name="trig_s")
                        nc.scalar.activation(trig_s[:], th[:], AF.Sin,
                                             bias=zero_c[:], scale=1.0)
                        trig_tiles[c + 4] = trig_s
                        tha = thp.tile([128, SC], F32, tag="tha", name="tha")
                        nc.vector.tensor_scalar(tha[:], th[:], -1.0, None,
                                                ALU.mult)
                        nc.vector.tensor_tensor(out=tha[:], in0=tha[:],
                                                in1=th[:], op=ALU.max)
                        trig_c = trigp.tile([128, SC], F32, tag="trig_c",
                                            name="trig_c")
                        nc.scalar.activation(trig_c[:], tha[:], AF.Sin,
                                             bias=halfpi_c[:], scale=-1.0)
                        trig_tiles[c] = trig_c

                        for dc in (c, c + 4):
                            pm = mmps.tile([128, SC], F32, tag="pm",
                                           name="pm")
                            for e in range(EC):
                                lhs = wab_sb[(b, e)][:, dc * 128:(dc + 1) * 128]
                                nc.tensor.matmul(pm[:, 0:HALF], lhs,
                                                 xts[e][:, 0:HALF],
                                                 start=(e == 0),
                                                 stop=(e == EC - 1))
                                nc.tensor.matmul(pm[:, HALF:SC], lhs,
                                                 xts[e][:, HALF:SC],
                                                 start=(e == 0),
                                                 stop=(e == EC - 1))
                            pt = pp.tile([128, SC], FP16, tag="p", name="p")
                            with tc.high_priority(offset=3000):
                                nc.vector.tensor_tensor(out=pt[:], in0=pm[:],
                                                        in1=trig_tiles[dc][:],
                                                        op=ALU.mult)
                            nc.tensor.matmul(lpA, ones_c[:],
                                             pt[:, 0:HALF],
                                             start=(n_acc == 0), stop=False)
                            nc.tensor.matmul(lpB, ones_c[:],
                                             pt[:, HALF:SC],
                                             start=(n_acc == 0), stop=False)
                            n_acc += 1
                    # unary half: u_w1 matmul + gelu + v-reduce
                    for dc in range(DC):
                        pm = mmps.tile([128, SC], F32, tag="pm", name="pm")
                        for e in range(EC):
                            lhs = uw1_sb[e][:, dc * 128:(dc + 1) * 128]
                            nc.tensor.matmul(pm[:, 0:HALF], lhs,
                                             xts[e][:, 0:HALF],
                                             start=(e == 0),
                                             stop=(e == EC - 1))
                            nc.tensor.matmul(pm[:, HALF:SC], lhs,
                                             xts[e][:, HALF:SC],
                                             start=(e == 0),
                                             stop=(e == EC - 1))
                        h = hp.tile([128, SC], FP16, tag="h", name="h")
                        with tc.high_priority(offset=3000):
                            nc.scalar.activation(h[:], pm[:], AF.Gelu,
                                                 bias=ub1[:, dc:dc + 1],
                                                 scale=1.0)
                        last = (dc == DC - 1)
                        nc.tensor.matmul(lpA, vv_sb[b][:, dc:dc + 1],
                                         h[:, 0:HALF], start=False, stop=last)
                        nc.tensor.matmul(lpB, vv_sb[b][:, dc:dc + 1],
                                         h[:, HALF:SC], start=False, stop=last)
                    orow = orowp.tile([1, SC], F32, tag="orow", name="orow")
                    nc.vector.tensor_copy(orow[:, 0:HALF], lpA)
                    nc.vector.tensor_copy(orow[:, HALF:SC], lpB)
                    nc.sync.dma_start(out_d[b, s0:s0 + SC], orow[:])

    _legalize_waits(nc)
    return nc


def _legalize_waits(nc, max_waits=1):
    """walrus on this stack accepts at most one sync-wait per instruction;
    move excess waits onto preceding same-engine NoOps."""
    import concourse.mybir as mybir

    ctr = 0
    for f in nc.m.functions:
        for bb in f.blocks:
            new_insts = []
            for inst in bb.instructions:
                si = inst.sync_info
                if si is not None and si.on_wait and len(si.on_wait) > max_waits:
                    waits = list(si.on_wait)
                    chunks = [waits[i:i + max_waits]
                              for i in range(0, len(waits), max_waits)]
                    for chunk in chunks[:-1]:
                        ctr += 1
                        nop = mybir.InstNoOp(name=f"waitnop-{ctr}", ins=[],
                                             outs=[])
                        nop.engine = inst.engine
                        nop.sync_info = mybir.SyncInfo(on_wait=list(chunk),
                                                       on_update=[])
                        new_insts.append(nop)
                    si.on_wait = chunks[-1]
                new_insts.append(inst)
            bb.instructions[:] = new_insts


def _host_prep(inputs):
    """All O(B)-sized math: summary, query, RoPE(q), weight folding."""
    f32 = np.float32
    x_ctx = np.asarray(inputs["x_ctx"], f32)
    pos_suf = np.asarray(inputs["pos_suf"])
    role_ctx = np.asarray(inputs["role_ctx"])
    role_tgt = np.asarray(inputs["role_tgt"])
    mask_ctx = np.asarray(inputs["mask_ctx"])
    ln_g = np.asarray(inputs["ln_g"], f32)
    ln_b = np.asarray(inputs["ln_b"], f32)
    hist_role_emb = np.asarray(inputs["hist_role_emb"], f32)
    tgt_role_emb = np.asarray(inputs["tgt_role_emb"], f32)
    start_state = np.asarray(inputs["start_state"], f32)
    wq = np.asarray(inputs["wq"], f32)
    wk = np.asarray(inputs["wk"], f32)
    u_w1 = np.asarray(inputs["u_w1"], f32)
    u_b1 = np.asarray(inputs["u_b1"], f32)
    u_w2 = np.asarray(inputs["u_w2"], f32)
    u_b2 = np.asarray(inputs["u_b2"], f32)

    lengths = mask_ctx.astype(np.int64).sum(1)
    last_idx = np.clip(lengths - 1, 0, None)
    last_x = x_ctx[np.arange(B), last_idx]
    last_role = role_ctx[np.arange(B), last_idx]
    cand = np.concatenate([last_x, hist_role_emb[last_role]], -1)
    summary = np.where((lengths > 0)[:, None], cand,
                       start_state[None, :]).astype(f32)
    q_in = np.concatenate([summary, tgt_role_emb[role_tgt]], -1).astype(f32)
    q = (q_in @ wq.T).astype(f32)                      # [B, D_MODEL]

    half = D_MODEL // 2
    inv_freq = (1.0 / (ROPE_BASE ** (np.arange(0, D_MODEL, 2,
                dtype=f32) / D_MODEL))).astype(f32)    # [512]
    anchor = pos_suf[:, :1].astype(f32) - 1.0          # [B, 1]
    ang = anchor * inv_freq[None, :]
    qc, qs = np.cos(ang).astype(f32), np.sin(ang).astype(f32)
    q1, q2 = q[:, :half], q[:, half:]
    qr1 = q1 * qc - q2 * qs
    qr2 = q1 * qs + q2 * qc

    scale = f32(1.0 / math.sqrt(D_MODEL))
    wk1, wk2 = wk[:half], wk[half:]                    # [512, D_EMB]
    wA = (qr1[:, :, None] * wk1[None] + qr2[:, :, None] * wk2[None]) * scale
    wB = (qr2[:, :, None] * wk1[None] - qr1[:, :, None] * wk2[None]) * scale
    wAB = (np.concatenate([wA, wB], axis=1) * ln_g[None, None, :]).astype(f32)
    # transpose to [e, d] (lhsT layout), fp16
    wabt = np.ascontiguousarray(wAB.transpose(0, 2, 1)).astype(np.float16)

    uw1g = (u_w1 * ln_g[None, :]).astype(f32)
    uw1t = np.ascontiguousarray(uw1g.T).astype(np.float16)  # [e, d]
    ub1e = (u_b1 + u_w1 @ ln_b).astype(f32)            # [1024]
    # attention-path layernorm-bias correction: zero when ln_b == 0.
    cab = wAB @ ln_b                                   # [B, 1024]
    assert np.abs(cab).max() < 1e-30, (
        "non-zero ln_b correction not supported by the device program")

    v = u_w2[role_tgt].astype(np.float16)              # [B, 1024]
    vb2 = u_b2[role_tgt].astype(f32)                   # [B]

    f_ext = np.concatenate([inv_freq, inv_freq])       # [1024]
    phase = np.concatenate([np.full(half, np.pi / 2, f32),
                            np.zeros(half, f32)])
    cons = np.zeros((128, 16), f32)
    cons[:, 0:8] = f_ext.reshape(8, 128).T
    cons[:, 8:16] = phase.reshape(8, 128).T
    ub1c = np.ascontiguousarray(ub1e.reshape(8, 128).T)  # [128, 8]
    vvc = np.ascontiguousarray(
        v.reshape(B, 8, 128).transpose(0, 2, 1))       # [B, 128, 8]

    pos_f = pos_suf.astype(f32)
    return dict(wabt=wabt, uw1t=uw1t, cons=cons, ub1c=ub1c, vvc=vvc,
                vb2=vb2, pos_f=pos_f)


def kernel(**inputs):
    from concourse.bass_utils import run_bass_kernel_spmd

    x_suf = np.asarray(inputs["x_suf"], np.float32)
    mask_suf = np.asarray(inputs["mask_suf"])
    role_tgt = np.asarray(inputs["role_tgt"])

    hp = _host_prep(inputs)

    if "prog" not in _PROGRAM_CACHE:
        _PROGRAM_CACHE["prog"] = _build_program()
    nc = _PROGRAM_CACHE["prog"]

    in_maps = []
    for c in range(N_CORES):
        bs = slice(c * BPC, (c + 1) * BPC)
        in_maps.append({
            "x": np.ascontiguousarray(x_suf[bs]),
            "pos": np.ascontiguousarray(hp["pos_f"][bs]),
            "wabt": np.ascontiguousarray(hp["wabt"][bs]),
            "uw1t": hp["uw1t"],
            "cons": hp["cons"],
            "ub1": hp["ub1c"],
            "vv": np.ascontiguousarray(hp["vvc"][bs]),
        })

    res = run_bass_kernel_spmd(nc, in_maps, list(range(N_CORES)))
    logits = np.empty((B, S), np.float32)
    for c in range(N_CORES):
        logits[c * BPC:(c + 1) * BPC] = res.results[c]["out"]

    logits += hp["vb2"][:, None]
    logits = np.where(mask_suf == 0, np.float32(-np.inf), logits)
    logits = logits.astype(np.float32)
    arg = np.argmax(logits, axis=-1).astype(np.int32)
    return logits, arg
